# revision 10
# baseline (speedup 1.0000x reference)
"""Trainium2 Bass kernel for nn_BaseSegHead (nms_detection).

Pipeline (B=2 images, F=100 feats/img, C=80 classes, 128x128 seg maps):
  Launch A (8 cores x 25 features): threshold seg maps, compute per-feature
    valid flag + tight box (first/last true row/col) via ones-matmul column
    sums + PE transpose row sums.
  Host glue (tiny): sigmoid scores, top-200, greedy NMS (200x200), top-50.
  Launch B (8 cores x 13 maps): gathered seg maps upsampled 128->512 with
    bilinear R @ X @ R^T on the TensorEngine, thresholded >0 to uint8 masks.
"""
import os
import sys

sys.path.insert(0, "/opt/trn_rl_repo")

import numpy as np

import concourse.bass as bass
import concourse.bacc as bacc
import concourse.mybir as mybir
from concourse.bass_utils import run_bass_kernel_spmd
from concourse.tile import TileContext

F32 = mybir.dt.float32
U8 = mybir.dt.uint8
ALU = mybir.AluOpType
AX = mybir.AxisListType

B, F, C = 2, 100, 80
FH = FW = 128
NMS_CAND = 200
NMS_THR = 0.65
MAX_SEGS = 50
OFFSET = 129.0
FEATS_PER_CORE = 25  # 200 features / 8 cores
MAPS_PER_CORE = 13  # ceil(2*50/8) with padding to 13*8 = 104 slots
N_CORES = 8


def _resize_matrix():
    # Exact jax.image.resize 'linear' 128->512 weights: resize is linear, so
    # resizing the identity yields the weight matrix R [512, 128].
    import jax

    with jax.default_device(jax.devices("cpu")[0]):
        eye = np.eye(FH, dtype=np.float32)
        r = jax.image.resize(eye, (512, FH), method="linear")
        return np.asarray(r, dtype=np.float32)


def _build_launch_a():
    nc = bacc.Bacc(None, target_bir_lowering=False, debug=False)
    seg = nc.dram_tensor("seg", [FEATS_PER_CORE, FH, FW], F32, kind="ExternalInput")
    iwp1000 = nc.dram_tensor("iwp1000", [1, FEATS_PER_CORE * FW], F32, kind="ExternalInput")
    iwp1 = nc.dram_tensor("iwp1", [1, FEATS_PER_CORE * FW], F32, kind="ExternalInput")
    ihp1000 = nc.dram_tensor("ihp1000", [128, FH], F32, kind="ExternalInput")
    ihp1 = nc.dram_tensor("ihp1", [128, FH], F32, kind="ExternalInput")
    ones = nc.dram_tensor("ones", [128, 1], F32, kind="ExternalInput")
    ident = nc.dram_tensor("ident", [128, 128], F32, kind="ExternalInput")
    xv_out = nc.dram_tensor("xv", [1, 3 * FEATS_PER_CORE], F32, kind="ExternalOutput")
    y_out = nc.dram_tensor("y", [FEATS_PER_CORE, 2], F32, kind="ExternalOutput")

    NFW = FEATS_PER_CORE * FW  # 3200

    with TileContext(nc) as tc:
        with (
            tc.tile_pool(name="sb", bufs=1) as pool,
            tc.tile_pool(name="ps", bufs=2, space="PSUM") as pp,
        ):
            ones_sb = pool.tile([128, 1], F32)
            nc.gpsimd.dma_start(out=ones_sb[:, :], in_=ones[:, :])
            id_sb = pool.tile([128, 128], F32)
            nc.gpsimd.dma_start(out=id_sb[:, :], in_=ident[:, :])
            iw1000_sb = pool.tile([1, NFW], F32)
            nc.gpsimd.dma_start(out=iw1000_sb[:, :], in_=iwp1000[:, :])
            iw1_sb = pool.tile([1, NFW], F32)
            nc.gpsimd.dma_start(out=iw1_sb[:, :], in_=iwp1[:, :])
            ih1000_sb = pool.tile([128, FH], F32)
            nc.gpsimd.dma_start(out=ih1000_sb[:, :], in_=ihp1000[:, :])
            ih1_sb = pool.tile([128, FH], F32)
            nc.gpsimd.dma_start(out=ih1_sb[:, :], in_=ihp1[:, :])

            # Load maps as [h, f, w] so h is the partition dim.
            S = pool.tile([128, NFW], F32)
            S3 = S[:, :].rearrange("p (f w) -> p f w", w=FW)
            nc.sync.dma_start(
                out=S3, in_=seg[:, :, :].rearrange("f h w -> h f w")
            )
            T = pool.tile([128, NFW], F32)
            nc.vector.tensor_scalar(T[:, :], S[:, :], 0.0, None, op0=ALU.is_gt)

            # Column projections: sum over h via ones-matmul -> [1, f*w].
            CA = pool.tile([1, NFW], F32)
            for c in range(7):
                n = min(512, NFW - c * 512)
                ps = pp.tile([1, 512], F32)
                nc.tensor.matmul(
                    ps[:, :n], ones_sb[:, :], T[:, c * 512 : c * 512 + n],
                    start=True, stop=True,
                )
                nc.vector.tensor_scalar(
                    CA[:, c * 512 : c * 512 + n], ps[:, :n], 0.0, None, op0=ALU.is_gt
                )

            CA3 = CA[:, :].rearrange("p (f w) -> p f w", w=FW)
            TMP = pool.tile([1, NFW], F32)
            TMP3 = TMP[:, :].rearrange("p (f w) -> p f w", w=FW)
            XV = pool.tile([1, 3 * FEATS_PER_CORE], F32)
            # x1 = min_w(iota_w + 1000*(1-col_any)) ; exact for valid feats
            nc.vector.scalar_tensor_tensor(
                TMP[:, :], CA[:, :], -1000.0, iw1000_sb[:, :], op0=ALU.mult, op1=ALU.add
            )
            nc.vector.tensor_reduce(
                XV[:, 0:FEATS_PER_CORE], TMP3, axis=AX.X, op=ALU.min
            )
            # x2 = max_w(col_any * (iota_w+1))
            nc.vector.scalar_tensor_tensor(
                TMP[:, :], CA[:, :], 1.0, iw1_sb[:, :], op0=ALU.mult, op1=ALU.mult
            )
            nc.vector.tensor_reduce(
                XV[:, FEATS_PER_CORE : 2 * FEATS_PER_CORE], TMP3, axis=AX.X, op=ALU.max
            )
            # valid = max_w(col_any)
            nc.vector.tensor_reduce(
                XV[:, 2 * FEATS_PER_CORE : 3 * FEATS_PER_CORE], CA3, axis=AX.X, op=ALU.max
            )

            # Row projections: sum over w per (h, f), then PE-transpose to [f, h].
            RS = pool.tile([128, 128], F32)
            nc.vector.memset(RS[:, :], 0.0)
            T3 = T[:, :].rearrange("p (f w) -> p f w", w=FW)
            nc.vector.tensor_reduce(
                RS[:, :FEATS_PER_CORE], T3, axis=AX.X, op=ALU.add
            )
            TP = pp.tile([128, 128], F32)
            nc.tensor.transpose(TP[:, :], RS[:, :], id_sb[:, :])
            RA = pool.tile([128, 128], F32)
            nc.vector.tensor_scalar(RA[:, :], TP[:, :], 0.0, None, op0=ALU.is_gt)

            TMPY = pool.tile([128, 128], F32)
            Y = pool.tile([FEATS_PER_CORE, 2], F32)
            nc.vector.scalar_tensor_tensor(
                TMPY[:, :], RA[:, :], -1000.0, ih1000_sb[:, :], op0=ALU.mult, op1=ALU.add
            )
            nc.vector.tensor_reduce(
                Y[:, 0:1], TMPY[:FEATS_PER_CORE, :], axis=AX.X, op=ALU.min
            )
            nc.vector.scalar_tensor_tensor(
                TMPY[:, :], RA[:, :], 1.0, ih1_sb[:, :], op0=ALU.mult, op1=ALU.mult
            )
            nc.vector.tensor_reduce(
                Y[:, 1:2], TMPY[:FEATS_PER_CORE, :], axis=AX.X, op=ALU.max
            )

            nc.sync.dma_start(out=xv_out[:, :], in_=XV[:, :])
            nc.sync.dma_start(out=y_out[:, :], in_=Y[:, :])
    nc.compile()
    return nc


def _build_launch_b():
    nc = bacc.Bacc(None, target_bir_lowering=False, debug=False)
    maps = nc.dram_tensor("maps", [MAPS_PER_CORE, FH, FW], F32, kind="ExternalInput")
    rt = nc.dram_tensor("rt", [128, 512], F32, kind="ExternalInput")
    ident = nc.dram_tensor("ident", [128, 128], F32, kind="ExternalInput")
    mout = nc.dram_tensor("mout", [MAPS_PER_CORE, 512, 512], U8, kind="ExternalOutput")

    with TileContext(nc) as tc:
        with (
            tc.tile_pool(name="sb", bufs=3) as pool,
            tc.tile_pool(name="cst", bufs=1) as cpool,
            tc.tile_pool(name="pst", bufs=2, space="PSUM") as pst,
            tc.tile_pool(name="psw", bufs=2, space="PSUM") as psw,
            tc.tile_pool(name="psz", bufs=2, space="PSUM") as psz,
        ):
            rt_sb = cpool.tile([128, 512], F32)
            nc.gpsimd.dma_start(out=rt_sb[:, :], in_=rt[:, :])
            id_sb = cpool.tile([128, 128], F32)
            nc.gpsimd.dma_start(out=id_sb[:, :], in_=ident[:, :])

            for m in range(MAPS_PER_CORE):
                x_sb = pool.tile([128, 128], F32, tag="xin")
                nc.sync.dma_start(out=x_sb[:, :], in_=maps[m, :, :])
                xt_ps = pst.tile([128, 128], F32, tag="xt_ps")
                nc.tensor.transpose(xt_ps[:, :], x_sb[:, :], id_sb[:, :])
                xt_sb = pool.tile([128, 128], F32, tag="xt")
                nc.vector.tensor_copy(xt_sb[:, :], xt_ps[:, :])
                # W = X @ R^T  ([128, 512])
                w_ps = psw.tile([128, 512], F32, tag="w_ps")
                nc.tensor.matmul(w_ps[:, :], xt_sb[:, :], rt_sb[:, :], start=True, stop=True)
                w_sb = pool.tile([128, 512], F32, tag="w")
                nc.vector.tensor_copy(w_sb[:, :], w_ps[:, :])
                # Z_b = R_b @ W  ([128, 512] x 4 blocks), threshold > 0
                for b in range(4):
                    z_ps = psz.tile([128, 512], F32, tag="z_ps")
                    nc.tensor.matmul(
                        z_ps[:, :], rt_sb[:, 128 * b : 128 * (b + 1)], w_sb[:, :],
                        start=True, stop=True,
                    )
                    m_sb = pool.tile([128, 512], U8, tag="m")
                    nc.vector.tensor_scalar(m_sb[:, :], z_ps[:, :], 0.0, None, op0=ALU.is_gt)
                    nc.sync.dma_start(
                        out=mout[m, 128 * b : 128 * (b + 1), :], in_=m_sb[:, :]
                    )
    nc.compile()
    return nc


def _nms_glue(xv, y, cls_logits):
    """Host glue: scores, top-200, greedy NMS, top-50. All tiny (O(200^2))."""
    import jax

    x1 = xv[:, 0]
    x2 = xv[:, 1]
    valid = xv[:, 2] > 0.5
    y1 = y[:, 0]
    y2 = y[:, 1]
    boxes = np.stack([x1, y1, x2, y2], axis=-1).astype(np.float32)  # [200, 4]

    with jax.default_device(jax.devices("cpu")[0]):
        sig = np.asarray(
            jax.nn.sigmoid(cls_logits.reshape(B, F, C + 1)[:, :, :-1]), np.float32
        )  # [B, F, C]

    out = []
    for img in range(B):
        v = valid[img * F : (img + 1) * F]
        bx = boxes[img * F : (img + 1) * F]
        scores = np.where(v[:, None], sig[img], np.float32(-1.0)).reshape(-1)
        order = np.argsort(-scores, kind="stable")[:NMS_CAND]
        cand_scores = scores[order]
        cand_feat = (order // C).astype(np.int32)
        cand_label = (order % C).astype(np.int32)
        cb = (bx[cand_feat] + (cand_label.astype(np.float32) * np.float32(OFFSET))[:, None]).astype(np.float32)
        cand_valid = cand_scores > -0.5
        # pairwise IoU in f32, matching reference op order
        area = ((cb[:, 2] - cb[:, 0]) * (cb[:, 3] - cb[:, 1])).astype(np.float32)
        lt = np.maximum(cb[:, None, :2], cb[None, :, :2])
        rb = np.minimum(cb[:, None, 2:], cb[None, :, 2:])
        wh = np.clip(rb - lt, np.float32(0.0), None)
        inter = (wh[..., 0] * wh[..., 1]).astype(np.float32)
        iou = inter / (area[:, None] + area[None, :] - inter + np.float32(1e-9))
        sup = iou > np.float32(NMS_THR)
        keep = np.zeros(NMS_CAND, dtype=bool)
        for i in range(NMS_CAND):
            keep[i] = cand_valid[i] and not np.any(sup[i, :i] & keep[:i])
        sel = np.where(keep, cand_scores, np.float32(-1.0))
        fin_idx = np.argsort(-sel, kind="stable")[:MAX_SEGS]
        fin_scores = sel[fin_idx]
        fin_feat = cand_feat[fin_idx]
        fin_label = cand_label[fin_idx]
        fin_valid = fin_scores > -0.5
        out.append((fin_label, fin_feat, fin_scores, fin_valid))
    return out


def kernel(cls_logits, seg_logits, batch_size, img_size):
    assert int(batch_size) == B and int(img_size) == 512
    cls_logits = np.ascontiguousarray(cls_logits, dtype=np.float32)
    seg_logits = np.ascontiguousarray(seg_logits, dtype=np.float32)

    core_ids = list(range(N_CORES))
    iota_w = np.tile(np.arange(FW, dtype=np.float32), FEATS_PER_CORE)[None, :]
    iota_h = np.broadcast_to(np.arange(FH, dtype=np.float32), (128, FH)).copy()
    consts_a = {
        "iwp1000": iota_w + np.float32(1000.0),
        "iwp1": iota_w + np.float32(1.0),
        "ihp1000": iota_h + np.float32(1000.0),
        "ihp1": iota_h + np.float32(1.0),
        "ones": np.ones((128, 1), np.float32),
        "ident": np.eye(128, dtype=np.float32),
    }

    nc_a = _build_launch_a()
    in_maps_a = [
        {"seg": seg_logits[c * FEATS_PER_CORE : (c + 1) * FEATS_PER_CORE], **consts_a}
        for c in core_ids
    ]
    trace = bool(os.environ.get("KTRACE"))
    import time as _time
    _t0 = _time.perf_counter()
    try:
        ra = run_bass_kernel_spmd(nc_a, in_maps_a, core_ids=core_ids, trace=trace)
    except ModuleNotFoundError:
        ra = run_bass_kernel_spmd(nc_a, in_maps_a, core_ids=core_ids)
    _t1 = _time.perf_counter()
    if trace:
        print(f"LAUNCH_A exec_time_ns: {ra.exec_time_ns} wall_s: {_t1 - _t0:.3f}")
    res_a = ra.results

    xv = np.concatenate(
        [r["xv"].reshape(3, FEATS_PER_CORE).T for r in res_a], axis=0
    )  # [200, 3] -> x1, x2, valid
    yy = np.concatenate([r["y"] for r in res_a], axis=0)  # [200, 2]

    per_img = _nms_glue(xv, yy, cls_logits)

    # Launch B: gather selected maps on host into per-core batches of 13.
    seg_b = seg_logits.reshape(B, F, FH, FW)
    slot_maps = np.full((N_CORES, MAPS_PER_CORE, FH, FW), -1.0, dtype=np.float32)
    for img in range(B):
        fin_label, fin_feat, fin_scores, fin_valid = per_img[img]
        for k in range(MAX_SEGS):
            slot = img * 52 + k  # 52 slots (4 cores x 13) per image
            c, s = divmod(slot, MAPS_PER_CORE)
            if fin_valid[k]:
                slot_maps[c, s] = seg_b[img, fin_feat[k]]

    rt = np.ascontiguousarray(_resize_matrix().T)  # [128, 512]
    nc_b = _build_launch_b()
    in_maps_b = [
        {"maps": slot_maps[c], "rt": rt, "ident": consts_a["ident"]}
        for c in core_ids
    ]
    _t2 = _time.perf_counter()
    try:
        rb = run_bass_kernel_spmd(nc_b, in_maps_b, core_ids=core_ids, trace=trace)
    except ModuleNotFoundError:
        rb = run_bass_kernel_spmd(nc_b, in_maps_b, core_ids=core_ids)
    _t3 = _time.perf_counter()
    if trace:
        print(f"LAUNCH_B exec_time_ns: {rb.exec_time_ns} wall_s: {_t3 - _t2:.3f}")
    res_b = rb.results

    masks = np.zeros((B, MAX_SEGS, 512, 512), dtype=bool)
    for img in range(B):
        for k in range(MAX_SEGS):
            slot = img * 52 + k
            c, s = divmod(slot, MAPS_PER_CORE)
            masks[img, k] = res_b[c]["mout"][s].astype(bool)

    labels = np.stack([p[0] for p in per_img]).astype(np.int32)
    scores = np.stack(
        [np.where(p[3], p[2], np.float32(0.0)) for p in per_img]
    ).astype(np.float32)
    valid = np.stack([p[3] for p in per_img])
    return labels, masks, scores, valid
